# revision 1
# baseline (speedup 1.0000x reference)
"""BatchedKiloNeRF Trainium2 kernel.

Strategy (expert-parallel, host routing, bf16, block-diagonal quads):
  - 4096 tiny MLPs ("experts"), 131072 points routed by model_indices.
  - PE throughput on this part is bound by the (LDWEIGHTS, MATMUL)
    instruction-pair issue rate (~34ns/pair, independent of stationary
    size), so experts are packed 4-per-matmul: quad q stacks experts
    b=0..3 on partition bands 32b with a block-diagonal stationary
    [128, 128]; one matmul streams C point-columns for 4 experts at once.
  - Host sorts experts by point count and packs groups of EPG=32 (8 quads)
    per core; points padded to the group capacity C (max count in the
    8*EPG-expert window). Hidden states are [128, 8C] bf16 SBUF tiles:
    partition band 32b = expert band, C-column segment q = quad.
  - PSUM: each step claims one full bank; group g uses banks (4g+j)%8,
    j = 0:L0, 1:L1, 2:sigma+rgb (shared slot: sigma at partitions 32-35,
    rgb at 0-11), 3:view. Concurrent matmuls in one bank share a row
    group (all start at partition 0), which the HW allows.
  - Block-diagonal L1/viewA stationaries would 4x the weight DMA, so they
    stream through NBUF rotating SBUF buffers that are memset to zero
    once; per group, 4 DMAs per layer overwrite only the diagonal blocks
    (the zeros persist). L0/viewB (K=16) and sigma/rgb (narrow M) blobs
    are small enough to ship dense from the host.
  - viewA (start) and viewB (stop) matmuls are emitted adjacently per
    quad: two accumulation groups open concurrently in the same (bank,
    partition band) lose the second matmul's contribution on HW.
  - Biases: L0/view biases ride in the matmul via a constant-1 input row;
    feat layer is folded into the view layer on the host. L1 bias is zero
    in practice (fast path: single relu copy); nonzero b1 falls back to
    per-quad tensor_scalar ops. sigma/rgb biases are added on host.
"""

import sys

import numpy as np
import ml_dtypes

BF16 = ml_dtypes.bfloat16

for _p in ("/opt/trn_rl_repo",):
    if _p not in sys.path:
        sys.path.append(_p)

NUM_MODELS = 4096
W = 32
N = 131072
NCORES = 8
EPG = 32               # experts per group per core (8 quads)
QPG = EPG // 4         # quads per group
NGROUPS = 512 // EPG
WIN = NCORES * EPG     # experts per capacity window
NBUF = 4               # rotating block-diag weight buffers per layer

# wblob per group [128, WBLOB_F] bf16 (dense, host-built):
#   sigma lhsT [0:4Q)     rows 32b+h, col 4q+b
#   rgb lhsT   [4Q:16Q)   rows 32b+h, col 12q+3b+r
#   b1 bias    [16Q:17Q)  rows 32b+h, col q
WBLOB_F = 17 * QPG
# sblob per group [16, 128Q]: w0aug lhsT, rows 4b+k, col 128q+32b+h
SBLOB_F = 128 * QPG
# l1d / vad DRAM streams: per (group, band b): [32, 32*QPG] diagonal blocks
BANK = 512
PIPE = 4


def _prep(x, model_indices, pts_w0, pts_b0, pts_w1, pts_b1,
          feat_w, feat_b, sigma_w, sigma_b, view_w, view_b, rgb_w, rgb_b):
    """Host-side routing + packing. Returns per-core device arrays and
    decode info."""
    x = np.asarray(x, np.float32)
    idx = np.asarray(model_indices).astype(np.int64)
    counts = np.bincount(idx, minlength=NUM_MODELS)

    expert_order = np.argsort(-counts, kind="stable")  # descending count
    caps = np.empty(NGROUPS, np.int64)
    for k in range(NGROUPS):
        win = expert_order[WIN * k:WIN * (k + 1)]
        c = int(counts[win].max())
        caps[k] = max(4, -(-c // 4) * 4)  # round up to multiple of 4, >=4
    assert caps.max() * QPG <= BANK, "group capacity exceeds one PSUM bank"
    colstart = np.concatenate([[0], np.cumsum(QPG * caps)])
    w_tot = int(colstart[-1])

    order_pts = np.argsort(idx, kind="stable")
    starts = np.concatenate([[0], np.cumsum(counts)])

    # fold the feat layer into the view layer on the host:
    #   view(h) = relu(Wv [feat(h); views] + bv)
    #           = relu((Wv[:, :32] @ Wf) h + WvB views + (bv + Wv[:, :32] bf))
    vb_fold = view_b + np.einsum("goh,gh->go", view_w[:, :, :W], feat_b)
    vwA_fold = np.einsum("gox,gxh->goh", view_w[:, :, :W], feat_w)
    # host-computed view-direction partial per point:
    #   vpart = Wv[:, 32:] @ views + vb_fold   (injected into PSUM on device
    #   via an identity-stationary matmul, so both view matmuls are K=128)
    vw_g = view_w[idx][:, :, W:]                      # [N, 32, 3]
    vpart_all = (np.einsum("nij,nj->ni", vw_g, x[:, 3:6])
                 + vb_fold[idx]).astype(np.float32)   # [N, 32]
    w0aug = np.concatenate(
        [np.transpose(pts_w0, (0, 2, 1)), pts_b0[:, None, :]], axis=1
    ).astype(np.float32)                      # [E, 4, 32] lhsT rows: xyz+bias
    w1T = np.transpose(pts_w1, (0, 2, 1)).astype(np.float32)    # [E,32,32]
    vwAT = np.transpose(vwA_fold, (0, 2, 1)).astype(np.float32)
    sigT = np.transpose(sigma_w, (0, 2, 1)).astype(np.float32)  # [E,32,1]
    rgbT = np.transpose(rgb_w, (0, 2, 1)).astype(np.float32)    # [E,32,3]
    b1 = np.asarray(pts_b1, np.float32)

    per_core = []
    decode = []
    for c in range(NCORES):
        gq = np.stack([expert_order[WIN * k + EPG * c: WIN * k + EPG * (c + 1)]
                       for k in range(NGROUPS)])  # [NGROUPS, EPG]

        wblob = np.zeros((NGROUPS, 128, WBLOB_F), np.float32)
        sblob = np.zeros((NGROUPS, 16, SBLOB_F), np.float32)
        bdl1 = np.zeros((NGROUPS, 128, 128 * QPG), np.float32)
        bdva = np.zeros((NGROUPS, 128, 128 * QPG), np.float32)
        xpts = np.zeros((16, w_tot), np.float32)
        vparts = np.zeros((128, w_tot), np.float32)
        xpts[3::4, :] = 1.0   # constant-1 rows for bias-in-matmul
        for k in range(NGROUPS):
            C = int(caps[k])
            col = int(colstart[k])
            for l in range(EPG):
                gid = int(gq[k, l])
                q, b = l // 4, l % 4
                wblob[k, 32 * b:32 * b + 32, 4 * q + b] = sigT[gid, :, 0]
                wblob[k, 32 * b:32 * b + 32, 4 * QPG + 12 * q + 3 * b:
                      4 * QPG + 12 * q + 3 * b + 3] = rgbT[gid]
                wblob[k, 32 * b:32 * b + 32, 16 * QPG + q] = b1[gid]
                sblob[k, 4 * b:4 * b + 4, 128 * q + 32 * b:
                      128 * q + 32 * b + 32] = w0aug[gid]
                bdl1[k, 32 * b:32 * b + 32,
                     128 * q + 32 * b:128 * q + 32 * b + 32] = w1T[gid]
                bdva[k, 32 * b:32 * b + 32,
                     128 * q + 32 * b:128 * q + 32 * b + 32] = vwAT[gid]
                cnt = int(counts[gid])
                pts = order_pts[starts[gid]:starts[gid] + cnt]
                cq = col + q * C
                if cnt:
                    xpts[4 * b:4 * b + 3, cq:cq + cnt] = x[pts, :3].T
                    vparts[32 * b:32 * b + 32, cq:cq + cnt] = vpart_all[pts].T
                decode.append((c, gid, pts, q, b, cq, cnt))
        per_core.append(dict(
            xpts=xpts.astype(BF16), vparts=vparts.astype(BF16),
            ident=np.eye(128, dtype=np.float32).astype(BF16),
            wblob=wblob.transpose(1, 0, 2).reshape(128, NGROUPS * WBLOB_F)
                       .astype(BF16),
            sblob=sblob.transpose(1, 0, 2).reshape(16, NGROUPS * SBLOB_F)
                       .astype(BF16),
            bdl1=bdl1.transpose(1, 0, 2)
                     .reshape(128, NGROUPS * 128 * QPG).astype(BF16),
            bdva=bdva.transpose(1, 0, 2)
                     .reshape(128, NGROUPS * 128 * QPG).astype(BF16)))

    b1_zero = not np.any(b1)
    return per_core, decode, caps, colstart, w_tot, b1_zero


def _build_nc(caps, w_tot, b1_zero):
    import concourse.mybir as mybir
    import concourse.tile as tile
    from concourse import bacc
    from contextlib import ExitStack

    f32 = mybir.dt.float32
    bf16 = mybir.dt.bfloat16
    RELU = mybir.ActivationFunctionType.Relu
    ADD = mybir.AluOpType.add
    MAX = mybir.AluOpType.max

    QW = 32 * QPG          # columns per (group, band) diag-block row
    BUFW = 128 * QPG       # block-diag buffer width per group

    nc = bacc.Bacc("TRN2", target_bir_lowering=False)
    xpts_d = nc.declare_dram_parameter("xpts", [16, w_tot], bf16, isOutput=False)
    vparts_d = nc.declare_dram_parameter("vparts", [128, w_tot], bf16,
                                         isOutput=False)
    ident_d = nc.declare_dram_parameter("ident", [128, 128], bf16,
                                        isOutput=False)
    wblob_d = nc.declare_dram_parameter("wblob", [128, NGROUPS * WBLOB_F], bf16,
                                        isOutput=False)
    sblob_d = nc.declare_dram_parameter("sblob", [16, NGROUPS * SBLOB_F], bf16,
                                        isOutput=False)
    bdl1_d = nc.declare_dram_parameter("bdl1", [128, NGROUPS * BUFW], bf16,
                                       isOutput=False)
    bdva_d = nc.declare_dram_parameter("bdva", [128, NGROUPS * BUFW], bf16,
                                       isOutput=False)
    out_d = nc.declare_dram_parameter("out", [16, w_tot], f32, isOutput=True)

    with tile.TileContext(nc) as tc, ExitStack() as ctx:
        const = ctx.enter_context(tc.tile_pool(name="const", bufs=1))
        hpool = ctx.enter_context(tc.tile_pool(name="h", bufs=4))
        pspool = ctx.enter_context(tc.tile_pool(name="ps", bufs=1, space="PSUM"))
        psall = pspool.tile([128, 8 * BANK], f32, tag="psall")
        # Global bank rotation; every step copies its slot out within the
        # step, so slot lifetime is one wave and 8 banks cover PIPE=4
        # windows without collisions.
        step_ctr = [0]

        def slot():
            bank = step_ctr[0] % 8
            step_ctr[0] += 1

            def mm_out(part_lo, m, q, C):
                base = bank * BANK + q * C
                return psall[part_lo:part_lo + m, base:base + C]

            def copy_src(part_lo, m, width):
                return psall[part_lo:part_lo + m,
                             bank * BANK:bank * BANK + width]

            return mm_out, copy_src

        xt = const.tile([16, w_tot], bf16)
        vpt = const.tile([128, w_tot], bf16)
        ident = const.tile([128, 128], bf16)
        nc.sync.dma_start(out=ident[:], in_=ident_d[:])
        wt_all = const.tile([128, NGROUPS * WBLOB_F], bf16)
        st_all = const.tile([16, NGROUPS * SBLOB_F], bf16)
        l1buf = const.tile([128, NGROUPS * BUFW], bf16)
        vabuf = const.tile([128, NGROUPS * BUFW], bf16)
        colstarts = np.concatenate([[0], np.cumsum(QPG * np.asarray(caps))])
        # Priority-ordered input DMAs, alternating between the sync and
        # gpsimd trigger queues: per group-range, ship exactly what its five
        # steps need, in step order, so group 0 starts computing after
        # ~400KB instead of ~9MB. First groups ship individually (critical
        # path), the rest in chunks of PIPE groups.
        ranges = [(g, g + 1) for g in range(PIPE)]
        ranges += [(a, min(a + PIPE, NGROUPS))
                   for a in range(PIPE, NGROUPS, PIPE)]
        dmas = []
        for a, b in ranges:
            xlo, xhi = colstarts[a], colstarts[b]
            dmas += [
                (xt[:, xlo:xhi], xpts_d[:, xlo:xhi]),
                (st_all[:, a * SBLOB_F:b * SBLOB_F],
                 sblob_d[:, a * SBLOB_F:b * SBLOB_F]),
                (l1buf[:, a * BUFW:b * BUFW], bdl1_d[:, a * BUFW:b * BUFW]),
                (wt_all[:, a * WBLOB_F:b * WBLOB_F],
                 wblob_d[:, a * WBLOB_F:b * WBLOB_F]),
                (vabuf[:, a * BUFW:b * BUFW], bdva_d[:, a * BUFW:b * BUFW]),
                (vpt[:, xlo:xhi], vparts_d[:, xlo:xhi]),
            ]
        engines = [nc.sync, nc.gpsimd]
        for i, (dst, src) in enumerate(dmas):
            engines[i % len(engines)].dma_start(out=dst, in_=src)
        otr_all = const.tile([12, w_tot], f32)
        ots_all = const.tile([4, w_tot], f32)

        def group_steps(g):
            C = int(caps[g])
            WC = QPG * C
            col = int(colstarts[g])
            wt = wt_all[:, g * WBLOB_F:(g + 1) * WBLOB_F]
            st = st_all[:, g * SBLOB_F:(g + 1) * SBLOB_F]
            l1w = l1buf[:, g * BUFW:(g + 1) * BUFW]
            vaw = vabuf[:, g * BUFW:(g + 1) * BUFW]
            state = {}

            def s_l0():
                mm0, cp0 = slot()
                for q in range(QPG):
                    nc.tensor.matmul(
                        out=mm0(0, 128, q, C),
                        lhsT=st[0:16, 128 * q:128 * q + 128],
                        rhs=xt[0:16, col + q * C:col + q * C + C],
                        start=True, stop=True, skip_group_check=True,
                        tile_position=(0, 0))
                state["cp0"] = cp0

            def c_l0():
                h1 = hpool.tile([128, WC], bf16, tag="h1")
                nc.scalar.activation(h1[:], state.pop("cp0")(0, 128, WC), RELU)
                state["h1"] = h1

            def s_l1():
                h1 = state.pop("h1")
                mm1, cp1 = slot()
                for q in range(QPG):
                    nc.tensor.matmul(
                        out=mm1(0, 128, q, C),
                        lhsT=l1w[:, 128 * q:128 * q + 128],
                        rhs=h1[:, q * C:q * C + C],
                        start=True, stop=True, skip_group_check=True,
                        tile_position=(0, 0))
                state["cp1"] = cp1

            def c_l1():
                cp1 = state.pop("cp1")
                h2 = hpool.tile([128, WC], bf16, tag="h2")
                if b1_zero:
                    nc.vector.tensor_scalar_max(h2[:], cp1(0, 128, WC), 0.0)
                else:
                    for q in range(QPG):
                        nc.vector.tensor_scalar(
                            out=h2[:, q * C:q * C + C],
                            in0=cp1(0, 128, WC)[:, q * C:q * C + C],
                            scalar1=wt[:, 16 * QPG + q:16 * QPG + q + 1],
                            scalar2=0.0, op0=ADD, op1=MAX)
                state["h2"] = h2

            def s_sigma():
                h2 = state["h2"]
                mms_, cps = slot()
                for q in range(QPG):
                    nc.tensor.matmul(
                        out=mms_(0, 4, q, C),
                        lhsT=wt[:, 4 * q:4 * q + 4],
                        rhs=h2[:, q * C:q * C + C],
                        start=True, stop=True, skip_group_check=True,
                        tile_position=(0, 0))
                state["cps"] = cps

            def c_sigma():
                eng = nc.scalar.copy if g % 2 else nc.vector.tensor_copy
                eng(ots_all[:, col:col + WC], state.pop("cps")(0, 4, WC))

            def s_view():
                h2 = state.pop("h2")
                mmv, cpv = slot()
                # Two uniform runs (all K=16, then all K=128): alternating
                # row sizes per instruction flushes the PE pipeline (~140ns
                # per matmul). viewB writes closed single-shots; viewA then
                # accumulates onto them (start=False) and closes. No two
                # accumulation groups are open concurrently (that corrupts
                # results on HW).
                for q in range(QPG):
                    nc.tensor.matmul(
                        out=mmv(0, 128, q, C),
                        lhsT=ident[:],
                        rhs=vpt[:, col + q * C:col + q * C + C],
                        start=True, stop=False, skip_group_check=True,
                        tile_position=(0, 0))
                    nc.tensor.matmul(
                        out=mmv(0, 128, q, C),
                        lhsT=vaw[:, 128 * q:128 * q + 128],
                        rhs=h2[:, q * C:q * C + C],
                        start=False, stop=True, skip_group_check=True,
                        tile_position=(0, 0))
                state["cpv"] = cpv

            def c_view():
                hv = hpool.tile([128, WC], bf16, tag="hv")
                nc.scalar.activation(hv[:], state.pop("cpv")(0, 128, WC), RELU)
                state["hv"] = hv

            def s_rgb():
                hv = state.pop("hv")
                mmr, cpr = slot()
                for q in range(QPG):
                    nc.tensor.matmul(
                        out=mmr(0, 12, q, C),
                        lhsT=wt[:, 4 * QPG + 12 * q:4 * QPG + 12 * q + 12],
                        rhs=hv[:, q * C:q * C + C],
                        start=True, stop=True, skip_group_check=True,
                        tile_position=(0, 0))
                state["cpr"] = cpr

            def c_rgb():
                eng = nc.vector.tensor_copy if g % 2 else nc.scalar.copy
                eng(otr_all[:, col:col + WC], state.pop("cpr")(0, 12, WC))

            return [(s_l0, c_l0), (s_l1, c_l1), (s_sigma, c_sigma),
                    (s_view, c_view), (s_rgb, c_rgb)]

        for base in range(0, NGROUPS, PIPE):
            window = [group_steps(g)
                      for g in range(base, min(base + PIPE, NGROUPS))]
            for stepi in range(5):
                for steps in window:
                    steps[stepi][0]()   # matmuls of the wave first
                for steps in window:
                    steps[stepi][1]()   # then the copies (waits satisfied)

        nc.sync.dma_start(out=out_d[0:12, :], in_=otr_all[0:12, :])
        nc.sync.dma_start(out=out_d[12:16, :], in_=ots_all[0:4, :])

    nc.compile()
    return nc


def _decode_out(results, decode, sigma_b, rgb_b):
    y = np.empty((N, 4), np.float32)
    outs = [np.asarray(r["out"]) for r in results]
    for (c, gid, pts, q, b, cq, cnt) in decode:
        if cnt == 0:
            continue
        o = outs[c]
        y[pts, 0:3] = o[3 * b:3 * b + 3, cq:cq + cnt].T + rgb_b[gid]
        y[pts, 3] = o[12 + b, cq:cq + cnt] + sigma_b[gid, 0]
    return y


def kernel(**inputs):
    from concourse.bass_utils import run_bass_kernel_spmd

    per_core, decode, caps, colstart, w_tot, b1_zero = _prep(**inputs)
    nc = _build_nc(caps, w_tot, b1_zero)
    in_maps = [per_core[c] for c in range(NCORES)]
    res = run_bass_kernel_spmd(nc, in_maps, list(range(NCORES)))
    return _decode_out(res.results, decode,
                       np.asarray(inputs["sigma_b"], np.float32),
                       np.asarray(inputs["rgb_b"], np.float32))


# ---------------------------------------------------------------------------
# numpy emulation of the device program (for layout validation in test.py)
def _emulate_core(arrs, caps, w_tot):
    arrs = {k: np.asarray(v, np.float32) for k, v in arrs.items()}
    xt = arrs["xpts"]
    vpt = arrs["vparts"]
    bdl1 = arrs["bdl1"]
    bdva = arrs["bdva"]
    out = np.zeros((16, w_tot), np.float32)
    col = 0
    for g in range(NGROUPS):
        C = int(caps[g])
        WC = QPG * C
        wt = arrs["wblob"][:, g * WBLOB_F:(g + 1) * WBLOB_F]
        st = arrs["sblob"][:, g * SBLOB_F:(g + 1) * SBLOB_F]
        l1w = bdl1[:, g * 128 * QPG:(g + 1) * 128 * QPG]
        vaw = bdva[:, g * 128 * QPG:(g + 1) * 128 * QPG]

        h1 = np.zeros((128, WC), np.float32)
        for q in range(QPG):
            h1[:, q * C:q * C + C] = (
                st[:, 128 * q:128 * q + 128].T
                @ xt[:, col + q * C:col + q * C + C])
        h1 = np.maximum(h1, 0)
        h2 = np.zeros((128, WC), np.float32)
        for q in range(QPG):
            h2[:, q * C:q * C + C] = (
                l1w[:, 128 * q:128 * q + 128].T @ h1[:, q * C:q * C + C]
                + wt[:, 16 * QPG + q:16 * QPG + q + 1])
        h2 = np.maximum(h2, 0)
        for q in range(QPG):
            out[12:16, col + q * C:col + q * C + C] = (
                wt[:, 4 * q:4 * q + 4].T @ h2[:, q * C:q * C + C])
        hv = np.zeros((128, WC), np.float32)
        for q in range(QPG):
            hv[:, q * C:q * C + C] = (
                vaw[:, 128 * q:128 * q + 128].T @ h2[:, q * C:q * C + C]
                + vpt[:, col + q * C:col + q * C + C])
        hv = np.maximum(hv, 0)
        for q in range(QPG):
            out[0:12, col + q * C:col + q * C + C] = (
                wt[:, 4 * QPG + 12 * q:4 * QPG + 12 * q + 12].T
                @ hv[:, q * C:q * C + C])
        col += WC
    return out


def kernel_emulated(**inputs):
    per_core, decode, caps, colstart, w_tot, b1_zero = _prep(**inputs)
    results = [{"out": _emulate_core(per_core[c], caps, w_tot)}
               for c in range(NCORES)]
    return _decode_out(results, decode,
                       np.asarray(inputs["sigma_b"], np.float32),
                       np.asarray(inputs["rgb_b"], np.float32))

